# revision 1
# baseline (speedup 1.0000x reference)
"""AxialAttention2D kernel for 8 TRN2 NeuronCores.

Sharding: data-parallel over B (B == 8 == n_cores). Each core processes one
full [C, H, W] image: both the height pass (attend along W for each row h)
and the width pass (attend along H for each column w), accumulating
(xh + xw) / 2 into an SBUF-resident fp32 accumulator. No collectives.

Inner structure: super-groups of SG=4 items (2 matmul sub-groups of 2),
scores row-tiled across 4 PSUM banks (concurrent row-tiled matmuls must
write different banks - HW constraint), one 2048-wide exp per super-group,
softmax row-sums via per-head N=512 ones-matmuls (col-tiled), projection
with 0.5/bias folded into host-side weights.

Self-contained: shapes are hardcoded (B=8, C=128, H=W=128, heads=4).
"""

import numpy as np
from contextlib import ExitStack

C = 128          # channels (= SBUF partitions)
L = 128          # attention sequence length (H or W)
HW = L * L       # flattened spatial size
HEADS = 4
HD = C // HEADS  # 32
SCALE = HD ** -0.5
SG = 4           # items per super-group
S_ITEMS = 40     # phase-0 H-items (overlap input DMA); multiple of SG
NCHUNK = 16
CHW = HW // NCHUNK  # 1024 columns per input chunk

_cache = {}

W_NAMES = ("wqT_h", "wkT_h", "wvT_h", "wpT_h", "wqT_w", "wkT_w", "wvT_w", "wpT_w")


def _build_nc():
    import concourse.bacc as bacc
    import concourse.tile as tile
    from concourse import mybir

    f32 = mybir.dt.float32
    bf16 = mybir.dt.bfloat16
    Exp = mybir.ActivationFunctionType.Exp
    Ident = mybir.ActivationFunctionType.Identity

    nc = bacc.Bacc(None, name="axial_attn")

    x_d = nc.dram_tensor("x", [C, HW], f32, kind="ExternalInput")
    w_d = {n: nc.dram_tensor(n, [C, C], bf16, kind="ExternalInput") for n in W_NAMES}
    bias_d = nc.dram_tensor("bias", [C, 1], f32, kind="ExternalInput")
    out_d = nc.dram_tensor("out", [C, HW], f32, kind="ExternalOutput")

    with ExitStack() as ctx:
        tc = ctx.enter_context(tile.TileContext(nc))
        singles = ctx.enter_context(tc.tile_pool(name="singles", bufs=1))
        big = ctx.enter_context(tc.tile_pool(name="big", bufs=1))
        work = ctx.enter_context(tc.tile_pool(name="work", bufs=3))
        nrm = ctx.enter_context(tc.tile_pool(name="nrm", bufs=2))
        # PSUM: s(4 banks, bufs=1) + qk(tag-shared q/k, bufs=2 -> 2 banks)
        #       + vap(tag-shared vT/av/rs/p, bufs=2 -> 2 banks) = 8 banks
        ps_s = ctx.enter_context(tc.tile_pool(name="ps_s", bufs=1, space="PSUM"))
        ps_qk = ctx.enter_context(tc.tile_pool(name="ps_qk", bufs=2, space="PSUM"))
        ps_vap = ctx.enter_context(tc.tile_pool(name="ps_vap", bufs=2, space="PSUM"))

        w_sb = {}
        for n in W_NAMES:
            w_sb[n] = singles.tile([C, C], bf16, tag=n, name=n)
            nc.sync.dma_start(out=w_sb[n][:], in_=w_d[n][:])
        bias_sb = singles.tile([C, 1], f32, tag="bias")
        nc.sync.dma_start(out=bias_sb[:], in_=bias_d[:])
        ones_sb = singles.tile([C, HD], bf16, tag="ones")
        nc.vector.memset(ones_sb[:], 1.0)

        x_sb = big.tile([C, HW], f32, tag="x_f32")
        xc = big.tile([C, HW], bf16, tag="x_bf16")
        acc = big.tile([C, HW], f32, tag="acc")

        # Strided views for the width pass: free dims become (w, h)
        xc_v = xc[:].rearrange("c (h w) -> c w h", w=L)
        acc_v = acc[:].rearrange("c (h w) -> c w h", w=L)

        def load_chunk(ci, eng=None):
            sl = slice(ci * CHW, (ci + 1) * CHW)
            nc.sync.dma_start(out=x_sb[:, sl], in_=x_d[:, sl])
            eng = eng or nc.gpsimd
            if eng is nc.scalar:
                eng.copy(out=xc[:, sl], in_=x_sb[:, sl])
            else:
                eng.tensor_copy(out=xc[:, sl], in_=x_sb[:, sl])

        def sgroup(passc, g0, mode):
            """Process items g0..g0+SG-1 of one pass.

            passc: 'h' (items are rows, attend along w) or 'w'.
            mode: 'init' -> acc = proj + bias      (phase-0 H-groups)
                  'w'    -> split add/init         (phase-1 W-groups)
                  'add'  -> acc += proj, DMA out   (phase-2 H-groups)
            """
            wq, wk, wv, wp = (w_sb[f"w{t}T_{passc}"] for t in ("q", "k", "v", "p"))
            SL = SG * L  # 512

            def xs_item(it):
                if passc == "h":
                    return xc[:, (g0 + it) * L:(g0 + it + 1) * L]
                return xc_v[:, g0 + it, :]

            # QKV.  q,k each as one N=512 matmul over all 4 items; vT per item.
            if passc == "h":
                rhs_qk = xc[:, g0 * L:(g0 + SG) * L]
            else:
                rhs_qk = xc_v[:, g0:g0 + SG, :]
            q_ps = ps_qk.tile([C, SL], f32, tag="qk", name="q_ps")
            nc.tensor.matmul(q_ps[:], wq[:], rhs_qk, start=True, stop=True)
            k_ps = ps_qk.tile([C, SL], f32, tag="qk", name="k_ps")
            nc.tensor.matmul(k_ps[:], wk[:], rhs_qk, start=True, stop=True)
            va = ps_vap.tile([C, SL], f32, tag="vap", name="vt_ps")
            for it in range(SG):
                nc.tensor.matmul(va[:, it * L:(it + 1) * L], xs_item(it), wv[:],
                                 start=True, stop=True)
            # layout: [q0..q3 | k0..k3 | vT0..vT3] (bf16)
            qkv_sb = work.tile([C, 3 * SL], bf16, tag="qkv_sb")
            nc.vector.tensor_copy(out=qkv_sb[:, 0:SL], in_=q_ps[:])
            nc.vector.tensor_copy(out=qkv_sb[:, SL:2 * SL], in_=k_ps[:])
            # vT copy for all 4 items at offset 1024 (ScalarE for balance)
            nc.scalar.copy(out=qkv_sb[:, 1024:1536], in_=va[:])

            # scores (transposed): sT_h[j, i] = sum_d k[d,j] q[d,i], row-tiled.
            # HW: concurrent row-tiled matmuls need different PSUM banks ->
            # head h -> bank h.  Layout: offset = h*512 + sub*256 + gl*128.
            s_ps = ps_s.tile([C, 2048], f32, tag="s")
            for it in range(SG):
                for h in range(HEADS):
                    off = h * 512 + it * L
                    qoff = it * L
                    koff = SL + it * L
                    nc.tensor.matmul(
                        s_ps[:, off:off + L],
                        qkv_sb[HD * h:HD * h + HD, koff:koff + L],
                        qkv_sb[HD * h:HD * h + HD, qoff:qoff + L],
                        start=True, stop=True, tile_position=(HD * h, 0))

            # exp in two bank-pair halves so s-banks free incrementally and
            # the next group's score matmuls can overlap the second half
            eT = work.tile([C, 2048], bf16, tag="eT")
            nc.scalar.activation(out=eT[:, 0:1024], in_=s_ps[:, 0:1024],
                                 func=Exp, scale=SCALE)
            nc.scalar.activation(out=eT[:, 1024:2048], in_=s_ps[:, 1024:2048],
                                 func=Exp, scale=SCALE)

            # A@V col-tiled per (item, head); rowsums via per-head N=512
            # ones-matmuls.  Column order of both: (it, i).
            av = ps_vap.tile([C, SL], f32, tag="vap", name="av_ps")
            rs = ps_vap.tile([C, SL], f32, tag="vap", name="rs_ps")
            for h in range(HEADS):
                nc.tensor.matmul(rs[HD * h:HD * h + HD, :], ones_sb[:],
                                 eT[:, h * 512:(h + 1) * 512],
                                 start=True, stop=True, tile_position=(0, HD * h))
            for it in range(SG):
                for h in range(HEADS):
                    esl = eT[:, h * 512 + it * L:h * 512 + (it + 1) * L]
                    nc.tensor.matmul(
                        av[HD * h:HD * h + HD, it * L:(it + 1) * L],
                        qkv_sb[:, 1024 + it * L + HD * h:1024 + it * L + HD * h + HD],
                        esl, start=True, stop=True, tile_position=(0, HD * h))

            rr = nrm.tile([C, SL], f32, tag="rr")
            nc.vector.reciprocal_approx_fast(out=rr[:], in_=rs[:])
            on = nrm.tile([C, SL], bf16, tag="on")
            nc.vector.tensor_mul(out=on[:], in0=av[:], in1=rr[:])

            p_ps = ps_vap.tile([C, SL], f32, tag="vap", name="p_ps")
            nc.tensor.matmul(p_ps[:], wp[:], on[:], start=True, stop=True)

            if mode == "init":
                nc.scalar.activation(out=acc[:, g0 * L:(g0 + SG) * L], in_=p_ps[:],
                                     func=Ident, bias=bias_sb[:], scale=1.0)
            elif mode == "w":
                accv = acc_v[:, g0:g0 + SG, :]
                pv = p_ps[:].rearrange("c (g l) -> c g l", g=SG)
                # items 0..S_ITEMS-1 were initialized in phase-0 -> add
                nc.vector.tensor_add(out=accv[:, :, 0:S_ITEMS],
                                     in0=pv[:, :, 0:S_ITEMS],
                                     in1=accv[:, :, 0:S_ITEMS])
                # rest: first write, carries the bias
                nc.scalar.activation(out=accv[:, :, S_ITEMS:L],
                                     in_=pv[:, :, S_ITEMS:L],
                                     func=Ident, bias=bias_sb[:], scale=1.0)
            else:  # "add"
                blk = acc[:, g0 * L:(g0 + SG) * L]
                nc.vector.tensor_add(out=blk, in0=p_ps[:], in1=blk)
                nc.sync.dma_start(out=out_d[:, g0 * L:(g0 + SG) * L], in_=blk)

        # ---- schedule ----
        # phase-0: first S_ITEMS height items, overlapping the input stream
        n_s_chunks = S_ITEMS * L // CHW  # 5
        for ci in range(n_s_chunks):
            load_chunk(ci)
        s_groups = list(range(0, S_ITEMS, SG))
        # late chunks: 5-8 cast on gpsimd right away (idle engine), the rest
        # on vector/scalar interleaved with late phase-0 groups
        for ci in (5, 6, 7, 8):
            load_chunk(ci)
        late = [(9, nc.vector), (10, nc.scalar), (11, nc.vector), (12, nc.scalar),
                (13, nc.vector), (14, nc.scalar), (15, nc.vector)]
        for i, g0 in enumerate(s_groups):
            sgroup("h", g0, "init")
            if i >= 3 and late:
                ci, eng = late.pop(0)
                load_chunk(ci, eng=eng)
        for ci, eng in late:
            load_chunk(ci, eng=eng)
        # phase-1: width pass (needs the full image)
        for g0 in range(0, L, SG):
            sgroup("w", g0, "w")
        # phase-2: remaining height items, streaming output
        s_out_chunks = list(range(n_s_chunks))  # cols 0..S_ITEMS*L final now
        for i, g0 in enumerate(range(S_ITEMS, L, SG)):
            sgroup("h", g0, "add")
            if i < len(s_out_chunks):
                ci = s_out_chunks[i]
                sl = slice(ci * CHW, (ci + 1) * CHW)
                nc.sync.dma_start(out=out_d[:, sl], in_=acc[:, sl])

    nc.finalize()
    return nc


def _get_nc():
    if "nc" not in _cache:
        _cache["nc"] = _build_nc()
    return _cache["nc"]


def _make_in_maps(x, wqkv_h, wproj_h, bproj_h, wqkv_w, wproj_w, bproj_w):
    import ml_dtypes
    bf = ml_dtypes.bfloat16
    x = np.asarray(x, dtype=np.float32)
    B = x.shape[0]

    def wT(w):
        return np.ascontiguousarray(np.asarray(w, np.float32).T)

    common = {
        "wqT_h": wT(wqkv_h[0:C]).astype(bf),
        "wkT_h": wT(wqkv_h[C:2 * C]).astype(bf),
        "wvT_h": wT(wqkv_h[2 * C:3 * C]).astype(bf),
        "wpT_h": (wT(wproj_h) * 0.5).astype(bf),
        "wqT_w": wT(wqkv_w[0:C]).astype(bf),
        "wkT_w": wT(wqkv_w[C:2 * C]).astype(bf),
        "wvT_w": wT(wqkv_w[2 * C:3 * C]).astype(bf),
        "wpT_w": (wT(wproj_w) * 0.5).astype(bf),
        "bias": (0.5 * (np.asarray(bproj_h, np.float32)
                        + np.asarray(bproj_w, np.float32))).reshape(C, 1),
    }
    return [
        {**common, "x": np.ascontiguousarray(x[b].reshape(C, HW))}
        for b in range(B)
    ]


def _run(in_maps, **kw):
    from concourse.bass_utils import run_bass_kernel_spmd
    nc = _get_nc()
    res = run_bass_kernel_spmd(nc, in_maps, core_ids=list(range(len(in_maps))), **kw)
    _cache["last_results"] = res
    return res


def kernel(x, wqkv_h, wproj_h, bproj_h, wqkv_w, wproj_w, bproj_w):
    in_maps = _make_in_maps(x, wqkv_h, wproj_h, bproj_h,
                            wqkv_w, wproj_w, bproj_w)
    res = _run(in_maps)
    out = np.stack([r["out"].reshape(C, L, L) for r in res.results], axis=0)
    return out.astype(np.float32)



# revision 6
# speedup vs baseline: 1.0820x; 1.0820x over previous
"""AxialAttention2D kernel for 8 TRN2 NeuronCores — v2.

Sharding: data-parallel over B (B == 8 == n_cores). Each core processes one
full [C, H, W] image. No collectives.

v2 restructure vs baseline:
- No fp32 accumulator. Phase 0: h-pass attention (no proj) for the first
  S_ITEMS rows, overlapped with the input stream; results stored as bf16
  `on_h_s`. Phase 1: w-pass attention (no proj) for all 128 cols, stored
  w-major as bf16 `on_w`. Phase 2: h-pass for remaining rows; every h-block
  runs proj_h + proj_w (strided rhs into on_w) accumulating in one PSUM
  tile, then a single bias-activation produces the fp32 output block
  (contiguous), DMA'd out.
- Normalization via a single tensor_tensor divide (no reciprocal+mul).
- q|k in one [C,1024] PSUM tile -> one 1024-col copy.
- Scores/exp split in item-pairs with double-buffered [C,1024] score PSUM
  so next-pair score matmuls overlap current exp.
- x staged through small rotating fp32 chunks (no persistent fp32 copy).

Self-contained: shapes hardcoded (B=8, C=128, H=W=128, heads=4).
"""

import numpy as np
from contextlib import ExitStack

C = 128          # channels (= SBUF partitions)
L = 128          # attention sequence length (H or W)
HW = L * L       # flattened spatial size
HEADS = 4
HD = C // HEADS  # 32
SCALE = HD ** -0.5
SG = 4           # items per super-group
S_ITEMS = 40     # phase-0 H-items (overlap input DMA); multiple of SG
NCHUNK = 16
CHW = HW // NCHUNK  # 1024 columns per input chunk (8 rows)

_cache = {}

W_NAMES = ("wqT_h", "wkT_h", "wvT_h", "wpT_h", "wqT_w", "wkT_w", "wvT_w", "wpT_w")


def _build_nc():
    import concourse.bacc as bacc
    import concourse.tile as tile
    from concourse import mybir
    from concourse.alu_op_type import AluOpType

    f32 = mybir.dt.float32
    bf16 = mybir.dt.bfloat16
    Exp = mybir.ActivationFunctionType.Exp
    Ident = mybir.ActivationFunctionType.Identity

    nc = bacc.Bacc(None, name="axial_attn")

    x_d = nc.dram_tensor("x", [C, HW], f32, kind="ExternalInput")
    w_d = {n: nc.dram_tensor(n, [C, C], bf16, kind="ExternalInput") for n in W_NAMES}
    bias_d = nc.dram_tensor("bias", [C, 1], f32, kind="ExternalInput")
    out_d = nc.dram_tensor("out", [C, HW], f32, kind="ExternalOutput")

    with ExitStack() as ctx:
        tc = ctx.enter_context(tile.TileContext(nc))
        singles = ctx.enter_context(tc.tile_pool(name="singles", bufs=1))
        big = ctx.enter_context(tc.tile_pool(name="big", bufs=1))
        xin = ctx.enter_context(tc.tile_pool(name="xin", bufs=3))
        nrm = ctx.enter_context(tc.tile_pool(name="nrm", bufs=2))
        work = ctx.enter_context(tc.tile_pool(name="work", bufs=3))
        work2 = ctx.enter_context(tc.tile_pool(name="work2", bufs=2))
        # PSUM: s (4 banks) + qk (2 banks) + vap (1 bank x2) = 8 banks
        ps_s = ctx.enter_context(tc.tile_pool(name="ps_s", bufs=1, space="PSUM"))
        ps_qk = ctx.enter_context(tc.tile_pool(name="ps_qk", bufs=1, space="PSUM"))
        ps_vap = ctx.enter_context(tc.tile_pool(name="ps_vap", bufs=2, space="PSUM"))

        w_sb = {}
        for n in W_NAMES:
            w_sb[n] = singles.tile([C, C], bf16, tag=n, name=n)
            nc.sync.dma_start(out=w_sb[n][:], in_=w_d[n][:])
        bias_sb = singles.tile([C, 1], f32, tag="bias")
        nc.sync.dma_start(out=bias_sb[:], in_=bias_d[:])
        ones_sb = singles.tile([C, HD], bf16, tag="ones")
        nc.vector.memset(ones_sb[:], 1.0)

        xc = big.tile([C, HW], bf16, tag="x_bf16")       # 4 MB
        on_w = big.tile([C, HW], bf16, tag="on_w")       # 4 MB, (w,h) layout
        on_h_s = big.tile([C, S_ITEMS * L], bf16, tag="on_h_s")

        # Strided views
        xc_v = xc[:].rearrange("c (h w) -> c w h", w=L)
        on_w_v = on_w[:].rearrange("c (w h) -> c h w", h=L)

        def load_chunk(ci, eng):
            sl = slice(ci * CHW, (ci + 1) * CHW)
            xt = xin.tile([C, CHW], f32, tag="xin", name="xin_t")
            nc.sync.dma_start(out=xt[:], in_=x_d[:, sl])
            if eng is nc.scalar:
                eng.copy(out=xc[:, sl], in_=xt[:])
            else:
                eng.tensor_copy(out=xc[:, sl], in_=xt[:])

        def attn_sg(passc, g0, store_ap):
            """Attention for items g0..g0+SG-1 of one pass; writes the
            normalized per-head output (pre-projection) to store_ap
            ([C, SG*L] bf16, cols ordered (item, seqpos))."""
            wq, wk, wv = (w_sb[f"w{t}T_{passc}"] for t in ("q", "k", "v"))
            SL = SG * L  # 512

            def xs_item(it):
                if passc == "h":
                    return xc[:, (g0 + it) * L:(g0 + it + 1) * L]
                return xc_v[:, g0 + it, :]

            if passc == "h":
                rhs_qk = xc[:, g0 * L:(g0 + SG) * L]
            else:
                rhs_qk = xc_v[:, g0:g0 + SG, :]

            qk = ps_qk.tile([C, 2 * SL], f32, tag="qk", name="qk_ps")
            nc.tensor.matmul(qk[:, 0:SL], wq[:], rhs_qk, start=True, stop=True)
            nc.tensor.matmul(qk[:, SL:2 * SL], wk[:], rhs_qk, start=True, stop=True)
            vt = ps_vap.tile([C, SL], f32, tag="vap", name="vt_ps")
            for it in range(SG):
                nc.tensor.matmul(vt[:, it * L:(it + 1) * L], xs_item(it), wv[:],
                                 start=True, stop=True)
            # layout: [q0..q3 | k0..k3 | vT0..vT3] (bf16)
            qkv = work.tile([C, 3 * SL], bf16, tag="qkv", name="qkv_sb")
            nc.vector.tensor_copy(out=qkv[:, 0:1024], in_=qk[:])
            nc.scalar.copy(out=qkv[:, 1024:1536], in_=vt[:])

            # scores (transposed): sT[j, i], head-major layout h*512 + it*128
            # so each head's row-tiled matmuls own one PSUM bank (HW rule).
            s = ps_s.tile([C, 2048], f32, tag="s", name="s_ps")
            for it in range(SG):
                qoff = it * L
                koff = SL + it * L
                for h in range(HEADS):
                    off = h * 512 + it * L
                    nc.tensor.matmul(
                        s[:, off:off + L],
                        qkv[HD * h:HD * h + HD, koff:koff + L],
                        qkv[HD * h:HD * h + HD, qoff:qoff + L],
                        start=True, stop=True, tile_position=(HD * h, 0))

            eT = work2.tile([C, 2048], bf16, tag="eT", name="eT_sb")
            av = ps_vap.tile([C, SL], f32, tag="vap", name="av_ps")
            rs = ps_vap.tile([C, SL], f32, tag="vap", name="rs_ps")
            nc.scalar.activation(out=eT[:, 0:1024], in_=s[:, 0:1024],
                                 func=Exp, scale=SCALE)
            nc.scalar.activation(out=eT[:, 1024:2048], in_=s[:, 1024:2048],
                                 func=Exp, scale=SCALE)
            for h in range(HEADS):
                nc.tensor.matmul(rs[HD * h:HD * h + HD, :], ones_sb[:],
                                 eT[:, h * 512:(h + 1) * 512],
                                 start=True, stop=True, tile_position=(0, HD * h))
            for it in range(SG):
                for h in range(HEADS):
                    esl = eT[:, h * 512 + it * L:h * 512 + (it + 1) * L]
                    nc.tensor.matmul(
                        av[HD * h:HD * h + HD, it * L:(it + 1) * L],
                        qkv[:, 1024 + it * L + HD * h:1024 + it * L + HD * h + HD],
                        esl, start=True, stop=True, tile_position=(0, HD * h))

            rr = nrm.tile([C, SL], f32, tag="rr", name="rr_sb")
            nc.vector.reciprocal_approx_fast(out=rr[:], in_=rs[:])
            nc.vector.tensor_mul(out=store_ap, in0=av[:], in1=rr[:])

        def proj_sg(g0, on_src):
            """proj_h(on_src) + proj_w(on_w slice) + bias -> out block."""
            p = ps_vap.tile([C, SG * L], f32, tag="vap", name="p_ps")
            nc.tensor.matmul(p[:], w_sb["wpT_h"][:], on_src,
                             start=True, stop=False)
            nc.tensor.matmul(p[:], w_sb["wpT_w"][:], on_w_v[:, g0:g0 + SG, :],
                             start=False, stop=True)
            outb = work2.tile([C, SG * L], f32, tag="outb", name="out_sb")
            nc.scalar.activation(out=outb[:], in_=p[:], func=Ident,
                                 bias=bias_sb[:], scale=1.0)
            nc.sync.dma_start(out=out_d[:, g0 * L:(g0 + SG) * L], in_=outb[:])

        # ---- schedule ----
        # phase-0: first S_ITEMS height items, overlapping the input stream
        n_s_chunks = S_ITEMS * L // CHW  # 5
        cast_engs = [nc.vector, nc.scalar, nc.gpsimd]
        for ci in range(n_s_chunks):
            load_chunk(ci, cast_engs[ci % 3])
        s_groups = list(range(0, S_ITEMS, SG))
        late = list(range(n_s_chunks, NCHUNK))  # chunks 5..15
        for i, g0 in enumerate(s_groups):
            attn_sg("h", g0, on_h_s[:, g0 * L:(g0 + SG) * L])
            if late:
                ci = late.pop(0)
                load_chunk(ci, cast_engs[ci % 3])
        for ci in late:
            load_chunk(ci, cast_engs[ci % 3])
        # phase-1: width pass (needs the full image); on_w is (w,h)-major
        for g0 in range(0, L, SG):
            attn_sg("w", g0, on_w[:, g0 * L:(g0 + SG) * L])
        # phase-2: remaining height items + all projections.
        # Interleave proj-only groups (rows < S_ITEMS) among attention groups
        # so the PE has dependency-free work during exp stalls.
        proj_only = list(range(0, S_ITEMS, SG))          # 10 groups
        fresh = list(range(S_ITEMS, L, SG))              # 22 groups
        for i, g0 in enumerate(fresh):
            onb = work2.tile([C, SG * L], bf16, tag="onb", name="onb_sb")
            attn_sg("h", g0, onb[:])
            proj_sg(g0, onb[:])
            if proj_only:
                g1 = proj_only.pop(0)
                proj_sg(g1, on_h_s[:, g1 * L:(g1 + SG) * L])
        for g1 in proj_only:
            proj_sg(g1, on_h_s[:, g1 * L:(g1 + SG) * L])

    nc.finalize()
    return nc


def _get_nc():
    if "nc" not in _cache:
        _cache["nc"] = _build_nc()
    return _cache["nc"]


def _make_in_maps(x, wqkv_h, wproj_h, bproj_h, wqkv_w, wproj_w, bproj_w):
    import ml_dtypes
    bf = ml_dtypes.bfloat16
    x = np.asarray(x, dtype=np.float32)
    B = x.shape[0]

    def wT(w):
        return np.ascontiguousarray(np.asarray(w, np.float32).T)

    common = {
        "wqT_h": wT(wqkv_h[0:C]).astype(bf),
        "wkT_h": wT(wqkv_h[C:2 * C]).astype(bf),
        "wvT_h": wT(wqkv_h[2 * C:3 * C]).astype(bf),
        "wpT_h": (wT(wproj_h) * 0.5).astype(bf),
        "wqT_w": wT(wqkv_w[0:C]).astype(bf),
        "wkT_w": wT(wqkv_w[C:2 * C]).astype(bf),
        "wvT_w": wT(wqkv_w[2 * C:3 * C]).astype(bf),
        "wpT_w": (wT(wproj_w) * 0.5).astype(bf),
        "bias": (0.5 * (np.asarray(bproj_h, np.float32)
                        + np.asarray(bproj_w, np.float32))).reshape(C, 1),
    }
    return [
        {**common, "x": np.ascontiguousarray(x[b].reshape(C, HW))}
        for b in range(B)
    ]


def _run(in_maps, **kw):
    from concourse.bass_utils import run_bass_kernel_spmd
    nc = _get_nc()
    res = run_bass_kernel_spmd(nc, in_maps, core_ids=list(range(len(in_maps))), **kw)
    _cache["last_results"] = res
    return res


def kernel(x, wqkv_h, wproj_h, bproj_h, wqkv_w, wproj_w, bproj_w):
    in_maps = _make_in_maps(x, wqkv_h, wproj_h, bproj_h,
                            wqkv_w, wproj_w, bproj_w)
    res = _run(in_maps)
    out = np.stack([r["out"].reshape(C, L, L) for r in res.results], axis=0)
    return out.astype(np.float32)


# revision 12
# speedup vs baseline: 1.1889x; 1.0989x over previous
"""AxialAttention2D kernel for 8 TRN2 NeuronCores — v3.

Sharding: data-parallel over B (B == 8 == n_cores). Each core processes one
full [C, H, W] image. No collectives.

v3 structure:
- Attention output computed TRANSPOSED: avT[i,(h,d)] = matmul(lhsT=eT_slice,
  rhs=[vT_h | ones]) with N=33 — the ones column makes the softmax row-sum a
  free by-product (col 32), killing the separate ones-matmul rowsums and
  shrinking the reciprocal to 16 cols. A PE transpose restores [(h,d), i]
  for the projection.
- No fp32 accumulator: phase-0 h-attention for the first S_ITEMS rows
  (overlapping the input stream) stores `on_h_s`; phase-1 w-attention stores
  w-major `on_w`; phase-2 h-blocks run proj_h + proj_w accumulating in one
  PSUM tile -> single bias-activation -> contiguous fp32 out block -> DMA.
- Global software pipeline: SG i+1's qk/vT matmuls+copies are emitted
  between SG i's first score block and its AV stage so the PE always has
  dependency-free work during exp stalls (p-state stays hot).
- Scores in [C,1024] head-pair blocks (each head row-tile owns a PSUM bank).

Self-contained: shapes hardcoded (B=8, C=128, H=W=128, heads=4).
"""

import numpy as np
from contextlib import ExitStack

C = 128          # channels (= SBUF partitions)
L = 128          # attention sequence length (H or W)
HW = L * L       # flattened spatial size
HEADS = 4
HD = C // HEADS  # 32
SCALE = HD ** -0.5
SG = 4           # items per super-group
SL = SG * L      # 512
NG = SG * HEADS  # 16 (it,h) groups per super-group
GW = HD + 1      # 33: av cols + rowsum col per group
S_ITEMS = 40     # phase-0 H-items (overlap input DMA); multiple of SG
NCHUNK = 16
CHW = HW // NCHUNK  # 1024 columns per input chunk (8 rows)

_cache = {}

W_NAMES = ("wqT_h", "wkT_h", "wvT_h", "wpT_h", "wqT_w", "wkT_w", "wvT_w", "wpT_w")


def _build_nc():
    import concourse.bacc as bacc
    import concourse.tile as tile
    from concourse import mybir

    f32 = mybir.dt.float32
    bf16 = mybir.dt.bfloat16
    Exp = mybir.ActivationFunctionType.Exp
    Ident = mybir.ActivationFunctionType.Identity

    nc = bacc.Bacc(None, name="axial_attn")

    x_d = nc.dram_tensor("x", [C, HW], f32, kind="ExternalInput")
    w_d = {n: nc.dram_tensor(n, [C, C], bf16, kind="ExternalInput") for n in W_NAMES}
    bias_d = nc.dram_tensor("bias", [C, 1], f32, kind="ExternalInput")
    ident_d = nc.dram_tensor("ident", [C, C], bf16, kind="ExternalInput")
    out_d = nc.dram_tensor("out", [C, HW], f32, kind="ExternalOutput")

    with ExitStack() as ctx:
        tc = ctx.enter_context(tile.TileContext(nc))
        singles = ctx.enter_context(tc.tile_pool(name="singles", bufs=1))
        big = ctx.enter_context(tc.tile_pool(name="big", bufs=1))
        xin = ctx.enter_context(tc.tile_pool(name="xin", bufs=3))
        nrm = ctx.enter_context(tc.tile_pool(name="nrm", bufs=2))
        work = ctx.enter_context(tc.tile_pool(name="work", bufs=3))
        work2 = ctx.enter_context(tc.tile_pool(name="work2", bufs=2))
        # PSUM (KB/partition): s 4 + qk 4 + avt ~2.1 + ring 2x2 = ~14.1 of 16
        ps_s = ctx.enter_context(tc.tile_pool(name="ps_s", bufs=1, space="PSUM"))
        ps_qk = ctx.enter_context(tc.tile_pool(name="ps_qk", bufs=1, space="PSUM"))
        ps_avt = ctx.enter_context(tc.tile_pool(name="ps_avt", bufs=1, space="PSUM"))
        ps_vap = ctx.enter_context(tc.tile_pool(name="ps_vap", bufs=2, space="PSUM"))

        w_sb = {}
        for n in W_NAMES:
            w_sb[n] = singles.tile([C, C], bf16, tag=n, name=n)
            nc.sync.dma_start(out=w_sb[n][:], in_=w_d[n][:])
        bias_sb = singles.tile([C, 1], f32, tag="bias")
        nc.sync.dma_start(out=bias_sb[:], in_=bias_d[:])
        ident_sb = singles.tile([C, C], bf16, tag="ident")
        nc.sync.dma_start(out=ident_sb[:], in_=ident_d[:])

        xc = big.tile([C, HW], bf16, tag="x_bf16")       # 4 MB
        on_w = big.tile([C, HW], bf16, tag="on_w")       # 4 MB, (w,h) layout
        on_h_s = big.tile([C, S_ITEMS * L], bf16, tag="on_h_s")

        xc_v = xc[:].rearrange("c (h w) -> c w h", w=L)
        on_w_v = on_w[:].rearrange("c (w h) -> c h w", h=L)

        def load_chunk(ci, eng):
            sl = slice(ci * CHW, (ci + 1) * CHW)
            xt = xin.tile([C, CHW], f32, tag="xin", name="xin_t")
            nc.sync.dma_start(out=xt[:], in_=x_d[:, sl])
            if eng is nc.scalar:
                eng.copy(out=xc[:, sl], in_=xt[:])
            else:
                eng.tensor_copy(out=xc[:, sl], in_=xt[:])

        def sg_front(passc, g0):
            """qkv matmuls + PSUM->SBUF copies for one super-group."""
            wq, wk, wv = (w_sb[f"w{t}T_{passc}"] for t in ("q", "k", "v"))

            def xs_item(it):
                if passc == "h":
                    return xc[:, (g0 + it) * L:(g0 + it + 1) * L]
                return xc_v[:, g0 + it, :]

            if passc == "h":
                rhs_qk = xc[:, g0 * L:(g0 + SG) * L]
            else:
                rhs_qk = xc_v[:, g0:g0 + SG, :]

            qk = ps_qk.tile([C, 2 * SL], f32, tag="qk", name="qk_ps")
            nc.tensor.matmul(qk[:, 0:SL], wq[:], rhs_qk, start=True, stop=True)
            nc.tensor.matmul(qk[:, SL:2 * SL], wk[:], rhs_qk, start=True, stop=True)
            vt = ps_vap.tile([C, SL], f32, tag="vap", name="vt_ps")
            for it in range(SG):
                nc.tensor.matmul(vt[:, it * L:(it + 1) * L], xs_item(it), wv[:],
                                 start=True, stop=True)
            qkv = work.tile([C, 2 * SL], bf16, tag="qkv", name="qkv_sb")
            nc.vector.tensor_copy(out=qkv[:], in_=qk[:])   # q|k, one 1024-col op
            # vte: per (it,h) group: [vT_h(it) (32) | 1.0] -> rhs of the AVT mm
            vte = work.tile([C, NG * GW], bf16, tag="vte", name="vte_sb")
            vte_g = vte[:].rearrange("c (g d) -> c g d", d=GW)
            nc.gpsimd.memset(vte_g[:, :, HD:GW], 1.0)
            nc.scalar.copy(out=vte_g[:, :, 0:HD],
                           in_=vt[:].rearrange("c (g d) -> c g d", d=HD))
            return qkv, vte

        def sg_back(passc, g0, fr, store_ap, fill=None):
            """Scores/exp/AV-T/normalize/transpose; writes [C, SL] bf16
            normalized per-head output (cols (item, seqpos)) to store_ap."""
            qkv, vte = fr
            eT = work2.tile([C, 2048], bf16, tag="eT", name="eT_sb")
            s = ps_s.tile([C, 1024], f32, tag="s", name="s_ps")

            def scores(hpair):
                for it in range(SG):
                    qoff = it * L
                    koff = SL + it * L
                    for hl in range(2):
                        h = 2 * hpair + hl
                        nc.tensor.matmul(
                            s[:, hl * 512 + it * L:hl * 512 + (it + 1) * L],
                            qkv[HD * h:HD * h + HD, koff:koff + L],
                            qkv[HD * h:HD * h + HD, qoff:qoff + L],
                            start=True, stop=True, tile_position=(HD * h, 0))

            # avt: two 512-col PSUM banks, 8 groups of 33 per bank so no
            # matmul output crosses a bank boundary.
            avt = ps_avt.tile([C, 1024], f32, tag="avt", name="avt_ps")

            def gcol(g):
                return (g // 8) * 512 + (g % 8) * GW

            def avts(hpair):
                for it in range(SG):
                    for hl in range(2):
                        h = 2 * hpair + hl
                        g = it * HEADS + h
                        nc.tensor.matmul(
                            avt[:, gcol(g):gcol(g) + GW],
                            eT[:, h * 512 + it * L:h * 512 + (it + 1) * L],
                            vte[:, g * GW:(g + 1) * GW],
                            start=True, stop=True)

            scores(0)
            nc.scalar.activation(out=eT[:, 0:1024], in_=s[:], func=Exp, scale=SCALE)
            nxt = fill() if fill is not None else None
            scores(1)
            nc.scalar.activation(out=eT[:, 1024:2048], in_=s[:], func=Exp, scale=SCALE)
            avts(0)
            avts(1)

            avt_j = avt[:].rearrange("c (p z) -> c p z", p=2)[:, :, 0:8 * GW] \
                .rearrange("c p (j d) -> c p j d", d=GW)
            rr = nrm.tile([C, NG], f32, tag="rr", name="rr_sb")
            nc.vector.reciprocal_approx_fast(
                out=rr[:].rearrange("c (p j) -> c p j", p=2),
                in_=avt_j[:, :, :, HD:GW].rearrange("c p j o -> c p (j o)"))
            onT = work2.tile([C, SL], bf16, tag="onT", name="onT_sb")
            rr_b = rr[:].rearrange("c (p j o) -> c p j o", p=2, o=1) \
                .broadcast_to([C, 2, 8, HD])
            nc.vector.tensor_mul(
                out=onT[:].rearrange("c (p j d) -> c p j d", p=2, d=HD),
                in0=avt_j[:, :, :, 0:HD], in1=rr_b)
            on_ps = ps_vap.tile([C, SL], bf16, tag="vap", name="on_ps")
            for it in range(SG):
                nc.tensor.transpose(on_ps[:, it * L:(it + 1) * L],
                                    onT[:, it * L:(it + 1) * L], ident_sb[:])
            nc.vector.tensor_copy(out=store_ap, in_=on_ps[:])
            return nxt

        def proj_sg(g0, on_src):
            """proj_h(on_src) + proj_w(on_w slice) + bias -> out block."""
            p = ps_vap.tile([C, SL], f32, tag="vap", name="p_ps")
            nc.tensor.matmul(p[:], w_sb["wpT_h"][:], on_src,
                             start=True, stop=False)
            nc.tensor.matmul(p[:], w_sb["wpT_w"][:], on_w_v[:, g0:g0 + SG, :],
                             start=False, stop=True)
            outb = work2.tile([C, SL], f32, tag="outb", name="out_sb")
            nc.scalar.activation(out=outb[:], in_=p[:], func=Ident,
                                 bias=bias_sb[:], scale=1.0)
            nc.sync.dma_start(out=out_d[:, g0 * L:(g0 + SG) * L], in_=outb[:])

        # ---- global pipelined schedule ----
        cast_engs = [nc.vector, nc.scalar, nc.gpsimd]
        n_s_chunks = S_ITEMS * L // CHW  # 5
        for ci in range(n_s_chunks):
            load_chunk(ci, cast_engs[ci % 3])

        # SG descriptors: (passc, g0, store_ap, proj_g0 | None)
        sgs = []
        for g0 in range(0, S_ITEMS, SG):
            sgs.append(("h", g0, on_h_s[:, g0 * L:(g0 + SG) * L], None))
        for g0 in range(0, L, SG):
            sgs.append(("w", g0, on_w[:, g0 * L:(g0 + SG) * L], None))
        fresh = list(range(S_ITEMS, L, SG))
        for g0 in fresh:
            sgs.append(("h", g0, None, g0))  # store to rotating onb, then proj

        late_chunks = list(range(n_s_chunks, NCHUNK))
        proj_only = list(range(0, S_ITEMS, SG))
        n_w0 = len([1 for s_ in sgs if s_[0] == "h" and s_[3] is None])  # 10

        fr = sg_front(sgs[0][0], sgs[0][1])
        for i, (passc, g0, store_ap, pj) in enumerate(sgs):
            if store_ap is None:
                store_ap = work2.tile([C, SL], bf16, tag="onb", name="onb_sb")[:]

            def fill(i=i):
                # All chunks must be cast before the first w-pass front
                # (emitted at i == n_w0 - 1) is issued.
                if late_chunks:
                    n_load = len(late_chunks) if i >= n_w0 - 1 else 1
                    for _ in range(n_load):
                        ci = late_chunks.pop(0)
                        load_chunk(ci, cast_engs[ci % 3])
                nxt = None
                if i + 1 < len(sgs):
                    nxt = sg_front(sgs[i + 1][0], sgs[i + 1][1])
                # sprinkle proj-only groups through phase-2 as PE filler
                if pj is not None and proj_only:
                    g1 = proj_only.pop(0)
                    proj_sg(g1, on_h_s[:, g1 * L:(g1 + SG) * L])
                return nxt

            fr = sg_back(passc, g0, fr, store_ap, fill=fill)
            if pj is not None:
                proj_sg(pj, store_ap)
        for ci in late_chunks:
            load_chunk(ci, cast_engs[ci % 3])
        for g1 in proj_only:
            proj_sg(g1, on_h_s[:, g1 * L:(g1 + SG) * L])

    nc.finalize()
    return nc


def _get_nc():
    if "nc" not in _cache:
        _cache["nc"] = _build_nc()
    return _cache["nc"]


def _make_in_maps(x, wqkv_h, wproj_h, bproj_h, wqkv_w, wproj_w, bproj_w):
    import ml_dtypes
    bf = ml_dtypes.bfloat16
    x = np.asarray(x, dtype=np.float32)
    B = x.shape[0]

    def wT(w):
        return np.ascontiguousarray(np.asarray(w, np.float32).T)

    common = {
        "wqT_h": wT(wqkv_h[0:C]).astype(bf),
        "wkT_h": wT(wqkv_h[C:2 * C]).astype(bf),
        "wvT_h": wT(wqkv_h[2 * C:3 * C]).astype(bf),
        "wpT_h": (wT(wproj_h) * 0.5).astype(bf),
        "wqT_w": wT(wqkv_w[0:C]).astype(bf),
        "wkT_w": wT(wqkv_w[C:2 * C]).astype(bf),
        "wvT_w": wT(wqkv_w[2 * C:3 * C]).astype(bf),
        "wpT_w": (wT(wproj_w) * 0.5).astype(bf),
        "bias": (0.5 * (np.asarray(bproj_h, np.float32)
                        + np.asarray(bproj_w, np.float32))).reshape(C, 1),
        "ident": np.eye(C, dtype=np.float32).astype(bf),
    }
    return [
        {**common, "x": np.ascontiguousarray(x[b].reshape(C, HW))}
        for b in range(B)
    ]


def _run(in_maps, **kw):
    from concourse.bass_utils import run_bass_kernel_spmd
    nc = _get_nc()
    res = run_bass_kernel_spmd(nc, in_maps, core_ids=list(range(len(in_maps))), **kw)
    _cache["last_results"] = res
    return res


def kernel(x, wqkv_h, wproj_h, bproj_h, wqkv_w, wproj_w, bproj_w):
    in_maps = _make_in_maps(x, wqkv_h, wproj_h, bproj_h,
                            wqkv_w, wproj_w, bproj_w)
    res = _run(in_maps)
    out = np.stack([r["out"].reshape(C, L, L) for r in res.results], axis=0)
    return out.astype(np.float32)
